# revision 3
# baseline (speedup 1.0000x reference)
"""MaxPoolingAggregator kernel for 8x TRN2 NeuronCores — v4.

Strategy (pure data parallel over nodes, 16384 nodes/core):

The baseline was DMA-descriptor-bound: the strided neigh load produced
512 B descriptors and the SBUF->SBUF xbar transposes produced 256 B
descriptors (~830K descriptors total, ~630us of per-DMA-engine time for
the transposes alone).  v4 removes both:

- neigh load: SWDGE cast f32->bf16 with partition = node, so each
  partition reads 25 consecutive DRAM rows (12.8 KB contiguous per
  descriptor).  SBUF layout nat[p, (t, d)] (neighbor-major).
- transpose: 25 PE transposes [128,128] bf16 per block (plus one for
  the self block), ~81ns each with FWL + LdW pull-ahead, written 8 per
  PSUM bank (bf16 packs at 2B in PSUM), drained bank-at-a-time on
  ACT/DVE.
- MLP: stationary W_mlp bf16, moving = strided AP over xt restoring
  (node, neighbor)-order; 4 matmuls at PSUM partition offsets
  0/32/64/96 pack one bank as [128, 400] so the 25-neighbor max-pool
  reduce runs on all 128 DVE lanes (2 reduces/block).
- stage 2: pooled result [(q,h), n] is the stationary operand against a
  block-diagonal replication of W_neigh (one matmul per 64 nodes);
  the self path accumulates into the same PSUM region with 4 small
  matmuls per 64 nodes (self vectors PE-transposed alongside the
  neighbor tiles).  Whole stage-2 in bf16 (fp32 PSUM accumulate).
- out = leaky(psum) stored natural-ish: [16, (blk, u, q, o)] staging,
  128 B descriptor stores (2 MB total, negligible).
"""

import sys

sys.path.insert(0, "/opt/trn_rl_repo")

import numpy as np

N_CORES = 8
N_TOTAL = 131072
NEIGH = 25
DIN = 128
DH = 32
DO = 32
SHARD = N_TOTAL // N_CORES      # 16384 nodes per core
BLK = 128                       # nodes per block
NBLK = SHARD // BLK             # 128 blocks
SLOTS = BLK * NEIGH             # 3200 neighbor rows per block
STORE_BATCH = 8                 # blocks per output store / self load
ALPHA = 0.02

_CACHE = {}


def _build(nblk=NBLK, hoist=True):
    import concourse.bass as bass
    import concourse.mybir as mybir
    from concourse.tile import TileContext

    fp32 = mybir.dt.float32
    bf16 = mybir.dt.bfloat16
    shard = nblk * BLK

    nc = bass.Bass()
    neigh = nc.dram_tensor("neigh", [shard, NEIGH, DIN], fp32, kind="ExternalInput")
    selfv = nc.dram_tensor("selfv", [shard, DIN], fp32, kind="ExternalInput")
    w_mlp = nc.dram_tensor("w_mlp", [DIN, DH], fp32, kind="ExternalInput")
    b_rep = nc.dram_tensor("b_rep", [128, 1], fp32, kind="ExternalInput")
    w_va = nc.dram_tensor("w_va", [DIN, DO], fp32, kind="ExternalInput")
    w_bd = nc.dram_tensor("w_bd", [128, 128], fp32, kind="ExternalInput")
    identity = nc.dram_tensor("identity", [128, 128], fp32, kind="ExternalInput")
    out = nc.dram_tensor("out", [shard, DO], fp32, kind="ExternalOutput")

    neigh_flat = neigh[:].rearrange("n j d -> (n j) d")   # [shard*25, 128]

    with TileContext(nc) as tc:
        with tc.tile_pool(name="const", bufs=1) as cpool, \
             tc.tile_pool(name="nat", bufs=5) as natpool, \
             tc.tile_pool(name="xt", bufs=4) as xtpool, \
             tc.tile_pool(name="sfb", bufs=3) as sfpool, \
             tc.tile_pool(name="sm", bufs=2) as smpool, \
             tc.tile_pool(name="ob", bufs=2) as opool, \
             tc.tile_pool(name="pst_big", bufs=4, space="PSUM") as pbig, \
             tc.tile_pool(name="ps_mlp", bufs=2, space="PSUM") as pmlp, \
             tc.tile_pool(name="ps_st2", bufs=2, space="PSUM") as pst2:

            # ---- constants (HWDGE f32 loads + DVE casts, so the Pool/
            # SWDGE queue starts the first neigh load immediately) ----
            wm_f = cpool.tile([DIN, DH], fp32)
            nc.sync.dma_start(wm_f[:], w_mlp[:])
            wv_f = cpool.tile([DIN, DO], fp32)
            nc.sync.dma_start(wv_f[:], w_va[:])
            wbd_f = cpool.tile([128, 128], fp32)
            nc.sync.dma_start(wbd_f[:], w_bd[:])
            ident_f = cpool.tile([128, 128], fp32)
            nc.sync.dma_start(ident_f[:], identity[:])
            bm = cpool.tile([128, 1], fp32)
            nc.sync.dma_start(bm[:], b_rep[:])
            wm = cpool.tile([DIN, DH], bf16)
            nc.vector.tensor_copy(wm[:], wm_f[:])
            wv = cpool.tile([DIN, DO], bf16)
            nc.vector.tensor_copy(wv[:], wv_f[:])
            wbd = cpool.tile([128, 128], bf16)
            nc.vector.tensor_copy(wbd[:], wbd_f[:])
            ident = cpool.tile([128, 128], bf16)
            nc.vector.tensor_copy(ident[:], ident_f[:])

            sfb = None
            out_tile = None
            for b in range(nblk):
                # ---- neighbor cast-load: partition = node ----
                # nat[p, 25t + ... ] free layout (t, d): per-partition
                # contiguous 12.8 KB DRAM read (25 rows of node 128b+p).
                nat = natpool.tile([128, NEIGH * DIN], bf16, tag="nat")
                src = neigh_flat[b * SLOTS:(b + 1) * SLOTS, :].rearrange(
                    "(p t) c -> p t c", p=128)
                nc.gpsimd.dma_start(
                    nat[:].rearrange("p (t c) -> p t c", t=NEIGH), src)

                # ---- self cast-load, batched 8 blocks ----
                if b % STORE_BATCH == 0:
                    sfb = sfpool.tile([128, STORE_BATCH * DIN], bf16, tag="sfb")
                    s0 = b * BLK
                    ssrc = selfv[s0:s0 + STORE_BATCH * BLK, :].rearrange(
                        "(k p) c -> p k c", p=128)
                    nc.gpsimd.dma_start(
                        sfb[:].rearrange("p (k c) -> p k c", k=STORE_BATCH), ssrc)
                sf = sfb[:, (b % STORE_BATCH) * DIN:(b % STORE_BATCH + 1) * DIN]

                # ---- PE transposes: 8+8+8 neigh, then [neigh24, self] ----
                # xt[d, 128t + p] = x[node 128b+p, neigh t, d]
                xt = xtpool.tile([128, SLOTS], bf16, tag="xt")
                sft = smpool.tile([128, DIN], bf16, tag="sft")
                for grp in range(3):
                    pt = pbig.tile([128, 1024], bf16, tag="ptb")
                    for i in range(8):
                        t = grp * 8 + i
                        nc.tensor.transpose(
                            pt[:, i * 128:(i + 1) * 128],
                            nat[:, t * DIN:(t + 1) * DIN], ident[:])
                    # drains: DVE reads packed bf16 from PSUM ~2x faster
                    # than ACT; alternate engines so consecutive groups
                    # drain in parallel (g0/g2 DVE, g1 ACT)
                    dst = xt[:, grp * 1024:(grp + 1) * 1024]
                    if grp == 1:
                        nc.scalar.copy(dst, pt[:])
                    else:
                        nc.vector.tensor_copy(dst, pt[:])
                ptl = pbig.tile([128, 1024], bf16, tag="ptb")
                nc.tensor.transpose(ptl[:, 0:128], nat[:, 24 * DIN:25 * DIN],
                                    ident[:])
                nc.tensor.transpose(ptl[:, 128:256], sf, ident[:])
                nc.vector.tensor_copy(xt[:, 24 * 128:25 * 128], ptl[:, 0:128])
                nc.scalar.copy(sft[:], ptl[:, 128:256])

                # ---- MLP matmuls: 2 banks x 4 partition-offset quarters ----
                # moving AP reorders xt columns to (node, neighbor):
                # mv[d, 25i + t] = xt[d, 128t + (16m + i)]
                xtv = xt[:].rearrange("d (t p) -> d p t", t=NEIGH)
                pool_sb = smpool.tile([128, 2 * 16], fp32, tag="pool")
                for u in range(2):
                    ps = pmlp.tile([128, 400], fp32, tag="mlp")
                    for q in range(4):
                        m = u * 4 + q
                        nc.tensor.matmul(
                            ps[32 * q:32 * (q + 1), :],
                            wm[:], xtv[:, 16 * m:16 * (m + 1), :],
                            start=True, stop=True,
                            tile_position=(0, 32 * q))
                    # max over the 25 neighbors (innermost), 128 lanes
                    nc.vector.tensor_reduce(
                        pool_sb[:, u * 16:(u + 1) * 16],
                        ps[:].rearrange("P (p j) -> P p j", j=NEIGH),
                        axis=mybir.AxisListType.X, op=mybir.AluOpType.max)

                # ---- bias + leaky on pooled [(q,h), (u,i)] -> bf16 ----
                pb = smpool.tile([128, 2 * 16], fp32, tag="pb")
                nc.vector.tensor_scalar(pb[:], pool_sb[:], bm[:], None,
                                        op0=mybir.AluOpType.add)
                hp = smpool.tile([128, 2 * 16], bf16, tag="hp")
                nc.vector.scalar_tensor_tensor(
                    hp[:], pb[:], ALPHA, pb[:],
                    op0=mybir.AluOpType.mult, op1=mybir.AluOpType.max)

                # ---- stage 2: per 64-node half u ----
                # st2[i, 128u + 32q + o] = out-pre for node 128b+64u+16q+i
                st2 = pst2.tile([16, 256], fp32, tag="st2")
                for u in range(2):
                    nc.tensor.matmul(
                        st2[:, 128 * u:128 * (u + 1)],
                        hp[:, 16 * u:16 * (u + 1)], wbd[:],
                        start=True, stop=False, skip_group_check=True)
                    for q in range(4):
                        nc.tensor.matmul(
                            st2[:, 128 * u + 32 * q:128 * u + 32 * (q + 1)],
                            sft[:, 64 * u + 16 * q:64 * u + 16 * (q + 1)],
                            wv[:], start=False, stop=(q == 3),
                            skip_group_check=True)

                # ---- final leaky -> staging, batched stores ----
                if b % STORE_BATCH == 0:
                    out_tile = opool.tile([16, STORE_BATCH * 256], fp32,
                                          tag="ob")
                sl = out_tile[:, (b % STORE_BATCH) * 256:
                              (b % STORE_BATCH + 1) * 256]
                t3 = smpool.tile([16, 256], fp32, tag="t3")
                nc.scalar.mul(t3[:], st2[:], ALPHA)
                nc.vector.tensor_tensor(sl, st2[:], t3[:],
                                        op=mybir.AluOpType.max)

                if b % STORE_BATCH == STORE_BATCH - 1:
                    b0 = b - (STORE_BATCH - 1)
                    dst = out[b0 * BLK:(b + 1) * BLK, :].rearrange(
                        "(k u q n) c -> n k u q c", k=STORE_BATCH, u=2, q=4)
                    nc.sync.dma_start(
                        dst, out_tile[:].rearrange(
                            "n (k u q c) -> n k u q c", k=STORE_BATCH, u=2,
                            q=4))
    if hoist:
        _hoist_excess_waits(nc)
    return nc


def _hoist_excess_waits(nc, limit=1):
    """Several ISA structs (Matmult among them) have too few sync-wait
    slots for what Tile emits.  Move excess waits onto standalone
    event-semaphore instructions on the same engine queue; the queue
    executes them in order ahead of the real instruction, so the waits
    are honored."""
    import concourse.mybir as mybir

    uid = [0]
    for f in nc.m.functions:
        for bb in f.blocks:
            new_insts = []
            for inst in bb.instructions:
                si = inst.sync_info
                if si is not None and len(si.on_wait) > limit:
                    excess = list(si.on_wait[limit - 1:]) if limit > 0 else \
                        list(si.on_wait)
                    si.on_wait = [w for w in si.on_wait if w not in excess]
                    for w in excess:
                        uid[0] += 1
                        carrier = mybir.InstEventSemaphore(
                            name=f"waitfix-{uid[0]}",
                            engine=inst.engine,
                            sync_info=mybir.SyncInfo(on_wait=[w], on_update=[]),
                        )
                        new_insts.append(carrier)
                new_insts.append(inst)
            bb.instructions = new_insts


def _host_weights(inputs):
    """Derived weight layouts (tiny, host-side)."""
    wng = np.asarray(inputs["W_neigh"], dtype=np.float32)     # [32, 32]
    b = np.asarray(inputs["b_mlp"], dtype=np.float32)         # [32]
    w_bd = np.zeros((128, 128), dtype=np.float32)
    for q in range(4):
        w_bd[32 * q:32 * (q + 1), 32 * q:32 * (q + 1)] = wng
    b_rep = np.tile(b, 4).reshape(128, 1).astype(np.float32)
    return w_bd, b_rep


def _get_nc():
    if "nc" not in _CACHE:
        _CACHE["nc"] = _build()
    return _CACHE["nc"]


def run(inputs, trace=False, **kwargs):
    from concourse.bass_utils import run_bass_kernel_spmd

    nc = _get_nc()
    ident = np.eye(128, dtype=np.float32)
    w_bd, b_rep = _host_weights(inputs)
    in_maps = []
    for c in range(N_CORES):
        sl = slice(c * SHARD, (c + 1) * SHARD)
        in_maps.append({
            "neigh": np.ascontiguousarray(inputs["neigh_vecs"][sl]),
            "selfv": np.ascontiguousarray(inputs["self_vecs"][sl]),
            "w_mlp": inputs["W_mlp"],
            "b_rep": b_rep,
            "w_va": inputs["W_va"],
            "w_bd": w_bd,
            "identity": ident,
        })
    res = run_bass_kernel_spmd(nc, in_maps, core_ids=list(range(N_CORES)),
                               trace=trace, **kwargs)
    outs = [res.results[c]["out"] for c in range(N_CORES)]
    full = np.concatenate(outs, axis=0)
    return full, res


def kernel(**inputs) -> np.ndarray:
    full, _ = run(inputs, trace=False)
    return full
